# revision 14
# baseline (speedup 1.0000x reference)
"""CapsuleLayer dynamic-routing kernel for 8 TRN2 NeuronCores.

Sharding: in_size (i) is split 8 ways (144 rows/core) for routing iterations
2-3; iteration 1 is batch-sharded.  u_hat (B,1152,10,16 = 189MB) is never
materialized: both the c-weighted sum (s_j) and the agreement update factor
through x and W:

    s_un[b, (d,j)]   = sum_{(i,u)} x[b,u,i] * (e[i,j] * W[i,j,d,u])
    A[(i,u), (d,j)]  = sum_b x[b,u,i] * v[b,j,d]
    u_vj1[i,j]       = (1/B) sum_{u,d} W[i,j,d,u] * A[(i,u),(d,j)]

Collective structure (the cost floor here is the per-collective constant, so
count is everything): iteration 1's coupling weights are exactly uniform
(softmax of ones), so s1 = x . (W/I) is a fixed linear map -- each core
computes s1 and v1 for its OWN 32 batches over the full contraction with a
prebaked W/I, and the only cross-core exchange is an AllGather of v1 (no
1.875x AllReduce tax).  Iterations 2-3 run i-sharded as before: AllReduce of
s2 (with the softmax denominator z riding in column 160), ReduceScatter of
s3, and each core squashes/emits only its own 32-batch output shard (gathered
host-side).  Key layouts: contraction index is (i*8+u) on partitions; the
160-wide capsule axis is d-major (d*10+j); s is produced in (b,(d,j))
orientation so squash needs no cross-partition reduction and v feeds the
A-matmul with no transposes; per-i-block sums/broadcasts (u_vj1, 1/z) are
constant 0/1-pattern matmuls.  All matmuls run in bf16 with fp32 PSUM
accumulation; exp/ln/copy/square stay in one ACT table so only one
LoadActFuncSet is ever issued.
"""

import os
import sys

import numpy as np

for _p in ("/opt/trn_rl_repo",):
    if _p not in sys.path and os.path.isdir(_p):
        sys.path.insert(0, _p)

import ml_dtypes

NCORES = 8
B, U, I = 256, 8, 1152
J, D = 10, 16
IL = I // NCORES        # 144 in_size rows per core
KL = IL * U             # 1152 local contraction length (i,u)
KT = KL // 128          # 9 partition tiles
JD = J * D              # 160
KTF = (I * U) // 128    # 72 full-contraction tiles (batch-sharded front)
BL = B // NCORES        # 32 own batches
W_END = KT * JD + 1
XT_END = W_END + KT * B
BF_COLS = XT_END + 2 * KL
F8_XO = KTF * BL
F8_COLS = F8_XO + KTF * JD                         # x-own | W-full (fp8 front)
F32_COLS = 256 + JD + 1                            # m8 | ones | sel10 | -ln(I)

_CACHE = {}


def _build_module():
    import concourse.bacc as bacc
    import concourse.mybir as mybir
    import concourse.tile as tile

    f32 = mybir.dt.float32
    bf16 = mybir.dt.bfloat16
    AF = mybir.ActivationFunctionType
    ALU = mybir.AluOpType
    AX = mybir.AxisListType

    # Force the act-table pass's first-match lookup to land every function
    # we use (Exp, Ln, Copy, Square) on the one table that covers them all,
    # so only a single LoadActFuncSet is ever emitted.  Table *ids* are
    # positional, so we only hide functions from other tables, never reorder.
    import concourse.hw_specs as hw_specs
    if not hasattr(bacc, "_orig_get_activation_tables"):
        bacc._orig_get_activation_tables = bacc.get_activation_tables

        def _patched_tables(arch):
            tabs = bacc._orig_get_activation_tables(arch)
            AF_ = mybir.ActivationFunctionType
            ours = {AF_.Exp, AF_.Ln, AF_.Copy, AF_.Square, AF_.Identity}
            out = {}
            for name, s in tabs.items():
                if name == "natural_log_exp_and_others":
                    out[name] = s
                else:
                    out[name] = s - ours
            return out

        bacc.get_activation_tables = _patched_tables

    nc = bacc.Bacc(
        "TRN2", target_bir_lowering=False, debug=False, num_devices=NCORES
    )

    bf_d = nc.declare_dram_parameter("bfin", [128, BF_COLS], bf16, isOutput=False)
    f8_d = nc.declare_dram_parameter("f8in", [128, F8_COLS], bf16, isOutput=False)
    f32_d = nc.declare_dram_parameter("f32in", [128, F32_COLS], f32, isOutput=False)
    out_d = nc.declare_dram_parameter("out", [B // NCORES, JD], f32, isOutput=True)

    with tile.TileContext(nc) as tc:
        with (
            tc.tile_pool(name="const", bufs=1) as cpool,
            tc.tile_pool(name="work", bufs=2) as wpool,
            tc.tile_pool(name="psum", bufs=1, space="PSUM") as ppool,
            tc.tile_pool(name="apsum", bufs=2, space="PSUM") as apool,
            tc.tile_pool(name="work3", bufs=3) as wpool3,
            tc.tile_pool(name="dram", bufs=3, space="DRAM") as dpool,
        ):
            # ---- persistent loads.  The batch-sharded front's x-own and
            # W-full stream first in fp8 (they gate s1; iteration 1 only sets
            # routing logits, never the output path, so fp8 noise is washed
            # out by the softmax over 1152 rows); the i-shard tensors ride
            # behind on both HWDGE queues and only need to land before the
            # post-AllGather phase. ----
            xo_sb = cpool.tile([128, KTF * BL], bf16)
            nc.sync.dma_start(xo_sb[:, :], f8_d[:, 0:F8_XO])
            xo = xo_sb[:, :].rearrange("p (t b) -> p t b", b=BL)
            wi_sb = cpool.tile([128, KTF * JD], bf16)
            WI_CH = 4
            wi_step = (KTF // WI_CH) * JD
            for ch in range(WI_CH):
                q = nc.scalar if ch % 2 else nc.sync
                q.dma_start(
                    wi_sb[:, ch * wi_step:(ch + 1) * wi_step],
                    f8_d[:, F8_XO + ch * wi_step:F8_XO + (ch + 1) * wi_step],
                )
            wi = wi_sb[:, :].rearrange("p (t n) -> p t n", n=JD)

            wsb_sb = cpool.tile([128, W_END], bf16)
            nc.sync.dma_start(wsb_sb[:, :], bf_d[:, 0:W_END])
            wsb = wsb_sb[:, 0:KT * JD].rearrange("p (t n) -> p t n", n=JD)
            ones8 = wsb_sb[:, KT * JD:W_END]
            f32_sb = cpool.tile([128, F32_COLS], f32)
            nc.scalar.dma_start(f32_sb[:, :], f32_d[:, :])
            xt_sb = cpool.tile([128, KT * B], bf16)
            nc.scalar.dma_start(xt_sb[:, :], bf_d[:, W_END:XT_END])
            xt = xt_sb[:, :].rearrange("p (t b) -> p t b", b=B)
            xb_sb = cpool.tile([128, 2 * KL], bf16)
            nc.sync.dma_start(xb_sb[:, :], bf_d[:, XT_END:BF_COLS])
            xb0 = xb_sb[:, 0:KL]
            xb1 = xb_sb[:, KL:2 * KL]
            m8 = f32_sb[:, 0:128]
            ones10 = f32_sb[0:J, 128:256]     # (10, 128) of ones
            sel10 = f32_sb[0:J, 256:256 + JD]  # sel10[j', d*J+j] = (j==j')
            negln_i = f32_sb[:, 256 + JD:256 + JD + 1]

            b_b = cpool.tile([128, KT, J], f32)     # b_ij replicated over u
            nc.vector.memset(b_b[:, :, :], 1.0)

            def wc_group(wc, e_b, g):
                nc.vector.tensor_tensor(
                    wc[:, 3 * g:3 * (g + 1), :].rearrange("p t (d j) -> p t d j", j=J),
                    wsb[:, 3 * g:3 * (g + 1), :].rearrange("p t (d j) -> p t d j", j=J),
                    e_b[:, 3 * g:3 * (g + 1), :].unsqueeze(2).broadcast_to([128, 3, D, J]),
                    ALU.mult,
                )

            def s_mms(s_ps, wc, t):
                # the two b-halves live in separate PSUM banks: a start=True
                # matmul clears its bank, so interleaved accumulation groups
                # must not share one
                s_ps0, s_ps1 = s_ps
                nc.tensor.matmul(
                    s_ps0[:, :], xt[:, t, 0:128], wc[:, t, :],
                    start=(t == 0), stop=(t == KT - 1),
                )
                nc.tensor.matmul(
                    s_ps1[:, :], xt[:, t, 128:B], wc[:, t, :],
                    start=(t == 0), stop=(t == KT - 1),
                )

            def z_mms(e_b):
                # z_loc[j] = sum_i e[i,j] = (1/8)*sum_partitions e_b, as a
                # (J,1) column; also keeps the PE warm between phases
                z_ps = ppool.tile([J, 1], f32, tag="z_ps")
                for t in range(KT):
                    nc.tensor.matmul(
                        z_ps[:, :], e_b[:, t, :], ones8[:, 0:1],
                        start=(t == 0), stop=(t == KT - 1),
                    )
                return z_ps

            def stage_and_collect(s_ps, z_ps, last):
                # stage [s | z] in SBUF: the z column is written into the
                # right partition blocks with tiny DVE copies so the two wide
                # DMAs carry everything; PSUM itself is not DMA-readable
                s_ps0, s_ps1 = s_ps
                cdt = f32 if last else bf16
                s_sb = wpool.tile([128, 2 * (JD + 1)], cdt, tag="s_sb")
                nc.scalar.copy(s_sb[:, 0:JD], s_ps0[:, :])
                nc.vector.tensor_copy(s_sb[:, JD + 1:2 * JD + 1], s_ps1[:, :])
                for r in range(4) if last else range(1):
                    nc.vector.tensor_copy(
                        s_sb[r * 32:r * 32 + J, JD:JD + 1], z_ps[:, :]
                    )
                    if last:
                        nc.vector.tensor_copy(
                            s_sb[r * 32:r * 32 + J, 2 * JD + 1:2 * JD + 2], z_ps[:, :]
                        )
                cc_in = dpool.tile([B, JD + 1], cdt, tag="cc_in")
                nc.sync.dma_start(
                    cc_in[:, :].rearrange("(c p) n -> p c n", p=128),
                    s_sb[:, :].rearrange("p (c n) -> p c n", n=JD + 1),
                )
                kind = "ReduceScatter" if last else "AllReduce"
                shape = [B // NCORES, JD + 1] if last else [B, JD + 1]
                cc_out = dpool.tile(shape, cdt, tag="cc3_out" if last else "cc_out", name="ccout")
                nc.gpsimd.collective_compute(
                    kind,
                    ALU.add,
                    replica_groups=[list(range(NCORES))],
                    ins=[cc_in.opt()],
                    outs=[cc_out.opt()],
                )
                return cc_out

            # ---- iteration 1, batch-sharded: c1 is exactly uniform (softmax
            # of ones), so s1 = x . (W/I) over the full (i,u) contraction for
            # this core's own 32 batches -- no exp, no z, no collective ----
            s1_ps = ppool.tile([BL, JD], f32, tag="s1_ps")
            for t in range(KTF):
                nc.tensor.matmul(
                    s1_ps[:, :], xo[:, t, :], wi[:, t, :],
                    start=(t == 0), stop=(t == KTF - 1),
                )
            # s1 = s_raw / I with the 1/I folded into the squash: the square
            # gets scale=1/I, and sqrt(m)/I = exp(0.5*ln(m) - ln(I)) folds the
            # remaining factor into the Exp bias -- zero extra instructions
            sq1 = wpool.tile([BL, JD], bf16, tag="sq1")
            nc.scalar.activation(sq1[:, :], s1_ps[:, :], AF.Square, scale=1.0 / I)
            msq1 = wpool.tile([BL, D], f32, tag="msq1")
            nc.vector.tensor_reduce(
                msq1[:, :],
                sq1[:, :].rearrange("p (d j) -> p d j", j=J),
                axis=AX.X,
                op=ALU.add,
            )
            ln1 = wpool.tile([BL, D], f32, tag="ln1")
            nc.scalar.activation(ln1[:, :], msq1[:, :], AF.Ln)
            rt1 = wpool.tile([BL, D], f32, tag="rt1")
            nc.scalar.activation(
                rt1[:, :], ln1[:, :], AF.Exp, scale=0.5, bias=negln_i[0:BL, 0:1]
            )
            dn1 = wpool.tile([BL, D], f32, tag="dn1")
            nc.vector.tensor_scalar_add(dn1[:, :], msq1[:, :], 1.0)
            rc1 = wpool.tile([BL, D], f32, tag="rc1")
            nc.vector.reciprocal(rc1[:, :], dn1[:, :])
            f1 = wpool.tile([BL, D], f32, tag="f1")
            nc.vector.tensor_mul(f1[:, :], rt1[:, :], rc1[:, :])
            v1 = wpool.tile([BL, JD], bf16, tag="v1")
            nc.vector.tensor_tensor(
                v1[:, :].rearrange("p (d j) -> p d j", j=J),
                s1_ps[:, :].rearrange("p (d j) -> p d j", j=J),
                f1[:, :].unsqueeze(2).broadcast_to([BL, D, J]),
                ALU.mult,
            )
            ag_in = dpool.tile([BL, JD], bf16, tag="ag_in")
            nc.sync.dma_start(ag_in[:, :], v1[:, :])
            ag_out = dpool.tile([B, JD], bf16, tag="ag_out", name="agout")
            nc.gpsimd.collective_compute(
                "AllGather",
                ALU.bypass,
                replica_groups=[list(range(NCORES))],
                ins=[ag_in.opt()],
                outs=[ag_out.opt()],
            )

            cc_out = None
            for it in range(2):
                last_cc = it == 1

                if it == 0:
                    # v1 arrives whole from the AllGather, already in the
                    # (b-on-partitions, (d,j)) layout the A-matmul wants
                    vt = wpool.tile([128, 2 * JD], bf16, tag="vt")
                    nc.sync.dma_start(
                        vt[:, :].rearrange("p (c n) -> p c n", n=JD),
                        ag_out[:, :].rearrange("(c p) n -> p c n", p=128),
                    )
                else:
                    # ---- post-AllReduce squash -> v ----
                    sgz = wpool.tile([128, 2 * (JD + 1)], bf16, tag="sgz")
                    nc.sync.dma_start(
                        sgz[:, :].rearrange("p (c n) -> p c n", n=JD + 1),
                        cc_out[:, :].rearrange("(c p) n -> p c n", p=128),
                    )
                    sg = sgz[:, :].rearrange("p (c n) -> p c n", n=JD + 1)[:, :, 0:JD]

                    # square the RAW s immediately (doesn't wait on the zinv
                    # chain); the 1/z^2 weighting rides in the magnitude
                    # product, and the plain 1/z folds into the final v
                    # product -- s/z is never materialized
                    sq = wpool.tile([128, 2 * JD], bf16, tag="sq")
                    nc.scalar.square(sq[:, :], sg)

                    # zinv and zinv^2 at (d,j) columns on all 128 partitions:
                    # recip the z column, scale sel10, then (K=10) ones-matmuls
                    zinv = wpool.tile([J, 2], f32, tag="zinv")
                    nc.vector.reciprocal(zinv[:, 0:1], sgz[0:J, JD:JD + 1])
                    nc.vector.tensor_mul(zinv[:, 1:2], zinv[:, 0:1], zinv[:, 0:1])
                    zsel = wpool.tile([J, 2 * JD], f32, tag="zsel")
                    nc.vector.tensor_scalar_mul(zsel[:, 0:JD], sel10[:, :], zinv[:, 0:1])
                    nc.vector.tensor_scalar_mul(zsel[:, JD:2 * JD], sel10[:, :], zinv[:, 1:2])
                    zbc_ps = ppool.tile([128, 2 * JD], f32, tag="zbc_ps")
                    nc.tensor.matmul(zbc_ps[:, :], ones10[:, :], zsel[:, :], start=True, stop=True)
                    zbc = zbc_ps[:, 0:JD]
                    zbc2 = zbc_ps[:, JD:2 * JD]

                    # mag_sq[b, d] = sum_j (s_raw/z)^2 via the zinv^2 weights
                    wsq = wpool.tile([128, 2 * JD], bf16, tag="wsq")
                    nc.vector.tensor_tensor(
                        wsq[:, :].rearrange("p (c n) -> p c n", n=JD),
                        sq[:, :].rearrange("p (c n) -> p c n", n=JD),
                        zbc2.unsqueeze(1).broadcast_to([128, 2, JD]),
                        ALU.mult,
                    )
                    msq = wpool.tile([128, 2 * D], f32, tag="msq")
                    nc.vector.tensor_reduce(
                        msq[:, :].rearrange("p (c d) -> p c d", d=D),
                        wsq[:, :].rearrange("p (c d j) -> p c d j", d=D, j=J),
                        axis=AX.X,
                        op=ALU.add,
                    )

                    # F = sqrt(m)/(1+m) on the tiny (128, 32) footprint;
                    # sqrt(m) = exp(0.5*ln(m)) keeps every ACT func in one table
                    lnm = wpool.tile([128, 2 * D], f32, tag="lnm")
                    nc.scalar.activation(lnm[:, :], msq[:, :], AF.Ln)
                    rt = wpool.tile([128, 2 * D], f32, tag="rt")
                    nc.scalar.activation(rt[:, :], lnm[:, :], AF.Exp, scale=0.5)
                    dn = wpool.tile([128, 2 * D], f32, tag="dn")
                    nc.vector.tensor_scalar_add(dn[:, :], msq[:, :], 1.0)
                    rc = wpool.tile([128, 2 * D], f32, tag="rc")
                    nc.vector.reciprocal(rc[:, :], dn[:, :])
                    f_t = wpool.tile([128, 2 * D], f32, tag="f_t")
                    nc.vector.tensor_mul(f_t[:, :], rt[:, :], rc[:, :])

                    # v = s_raw * F * (1/z); F broadcast over j, zinv over
                    # (b, d); v lands directly in the (b, (d,j)) layout the
                    # A-matmul needs -- no transposes
                    vt1 = wpool.tile([128, 2 * JD], bf16, tag="vt1")
                    for ch in range(2):
                        nc.vector.tensor_tensor(
                            vt1[:, ch * JD:(ch + 1) * JD].rearrange("p (d j) -> p d j", j=J),
                            sgz[:, ch * (JD + 1):ch * (JD + 1) + JD].rearrange("p (d j) -> p d j", j=J),
                            f_t[:, ch * D:(ch + 1) * D].unsqueeze(2).broadcast_to([128, D, J]),
                            ALU.mult,
                        )
                    vt = wpool.tile([128, 2 * JD], bf16, tag="vt")
                    nc.vector.tensor_tensor(
                        vt[:, :].rearrange("p (c n) -> p c n", n=JD),
                        vt1[:, :].rearrange("p (c n) -> p c n", n=JD),
                        zbc.unsqueeze(1).broadcast_to([128, 2, JD]),
                        ALU.mult,
                    )
                vb0 = vt[:, 0:JD]
                vb1 = vt[:, JD:2 * JD]

                # ---- fused: A-path group g immediately feeds the next
                # iteration's e/Wc/s-matmuls for those k-tiles ----
                e_b = wpool.tile([128, KT, J], bf16, tag="e_b")
                wc = wpool.tile([128, KT, JD], bf16, tag="wc")
                s_ps = (
                    ppool.tile([128, JD], f32, tag="s_ps0", name="s_ps0"),
                    ppool.tile([128, JD], f32, tag="s_ps1", name="s_ps1"),
                )
                r_t = wpool.tile([128, KT, J], f32, tag="r_t")
                uv_ps = ppool.tile([128, KT * J], f32, tag="uv_ps")
                for g in range(3):
                    a_ps = apool.tile([128, 3 * JD], f32, tag="a_ps")
                    for tt in range(3):
                        t = g * 3 + tt
                        nc.tensor.matmul(
                            a_ps[:, tt * JD:(tt + 1) * JD],
                            xb0[:, t * 128:(t + 1) * 128], vb0,
                            start=True, stop=False,
                        )
                        nc.tensor.matmul(
                            a_ps[:, tt * JD:(tt + 1) * JD],
                            xb1[:, t * 128:(t + 1) * 128], vb1,
                            start=False, stop=True,
                        )
                    a_sb = wpool3.tile([128, 3 * JD], bf16, tag="a_sb")
                    nc.scalar.copy(a_sb[:, :], a_ps[:, :])
                    p_t = wpool3.tile([128, 3 * JD], bf16, tag="p_t")
                    nc.vector.tensor_tensor(
                        p_t[:, :],
                        wsb[:, g * 3:(g + 1) * 3, :].rearrange("p a n -> p (a n)"),
                        a_sb[:, :],
                        ALU.mult,
                    )
                    nc.vector.tensor_reduce(
                        r_t[:, g * 3:(g + 1) * 3, :],
                        p_t.rearrange("p (a d j) -> p a j d", d=D, j=J),
                        axis=AX.X,
                        op=ALU.add,
                    )
                # single 90-column matmul: uv[(i,u-blocks), (t j)] via the
                # constant block-averaging matrix m8
                nc.tensor.matmul(
                    uv_ps[:, :], m8[:, :],
                    r_t[:, :, :].rearrange("p t j -> p (t j)"),
                    start=True, stop=True,
                )
                nc.vector.tensor_add(
                    b_b[:, :, :], b_b[:, :, :],
                    uv_ps.rearrange("p (t j) -> p t j", j=J),
                )
                nc.scalar.activation(e_b[:, :, :], b_b[:, :, :], AF.Exp)
                for g in range(3):
                    wc_group(wc, e_b, g)
                for t in range(KT):
                    s_mms(s_ps, wc, t)
                cc_out = stage_and_collect(s_ps, z_mms(e_b), last=last_cc)

            # ---- post-ReduceScatter shard squash -> out ----
            sg3z = wpool.tile([32, JD + 1], f32, tag="sg3z")
            nc.sync.dma_start(sg3z[:, :], cc_out[0:32, 0:JD + 1])
            zinv3 = wpool.tile([J, 1], f32, tag="zinv3")
            nc.vector.reciprocal(zinv3[:, :], sg3z[0:J, JD:JD + 1])
            zsel3 = wpool.tile([J, JD], f32, tag="zsel3")
            nc.vector.tensor_scalar_mul(zsel3[:, :], sel10[:, :], zinv3[:, 0:1])
            zbc3 = ppool.tile([32, JD], f32, tag="zbc_ps", name="zbc3")
            nc.tensor.matmul(zbc3[:, :], ones10[:, 0:32], zsel3[:, :], start=True, stop=True)
            sn3 = wpool.tile([32, JD], f32, tag="sn3")
            nc.vector.tensor_mul(sn3[:, :], sg3z[0:32, 0:JD], zbc3[:, :])
            sq3 = wpool.tile([32, JD], bf16, tag="sq3")
            nc.scalar.square(sq3[:, :], sn3[:, :])
            msq3 = wpool.tile([32, D], f32, tag="msq3")
            nc.vector.tensor_reduce(
                msq3[:, :],
                sq3[:, :].rearrange("p (d j) -> p d j", j=J),
                axis=AX.X,
                op=ALU.add,
            )
            ln3 = wpool.tile([32, D], f32, tag="ln3")
            nc.scalar.activation(ln3[:, :], msq3[:, :], AF.Ln)
            rt3 = wpool.tile([32, D], f32, tag="rt3")
            nc.scalar.activation(rt3[:, :], ln3[:, :], AF.Exp, scale=0.5)
            dn3 = wpool.tile([32, D], f32, tag="dn3")
            nc.vector.tensor_scalar_add(dn3[:, :], msq3[:, :], 1.0)
            rc3 = wpool.tile([32, D], f32, tag="rc3")
            nc.vector.reciprocal(rc3[:, :], dn3[:, :])
            f3 = wpool.tile([32, D], f32, tag="f3")
            nc.vector.tensor_mul(f3[:, :], rt3[:, :], rc3[:, :])
            v3 = wpool.tile([32, JD], f32, tag="v3")
            nc.vector.tensor_tensor(
                v3[:, :].rearrange("p (d j) -> p d j", j=J),
                sn3[:, :].rearrange("p (d j) -> p d j", j=J),
                f3[:, :].unsqueeze(2).broadcast_to([32, D, J]),
                ALU.mult,
            )
            nc.sync.dma_start(out_d[:, :], v3[:, :])

    nc.finalize()
    return nc


def _f32_blob():
    blob = np.zeros((128, F32_COLS), np.float32)
    blob[:, 0:128] = np.kron(np.eye(16, dtype=np.float32), np.ones((8, 8), np.float32)) / B
    blob[0:J, 128:256] = 1.0
    blob[0:J, 256:256 + JD] = np.tile(np.eye(J, dtype=np.float32), (1, D))
    blob[:, 256 + JD] = -np.log(float(I))
    return blob


def _prep_in_maps(x, W):
    x = np.asarray(x, np.float32)
    W = np.asarray(W, np.float32)
    Wm = W[0]
    f32_blob = _f32_blob()
    # full-contraction W for the batch-sharded first iteration (the 1/I is
    # folded into the on-core squash so fp8 doesn't underflow), shared by all
    # cores: rows (i*8+u), cols (d*10+j)
    wif = Wm.transpose(0, 3, 2, 1).reshape(I * U, JD)
    wif_t = wif.reshape(KTF, 128, JD).transpose(1, 0, 2).reshape(128, KTF * JD)
    in_maps = []
    for c in range(NCORES):
        sl = slice(c * IL, (c + 1) * IL)
        xs = x[:, :, sl]                                            # (B, U, IL)
        xt = np.ascontiguousarray(xs.transpose(2, 1, 0).reshape(KL, B))
        xb = xt.T
        w = Wm[sl].transpose(0, 3, 2, 1).reshape(KL, JD)   # cols = (d, j)
        xown = x[c * BL:(c + 1) * BL].transpose(2, 1, 0).reshape(I * U, BL)
        bf = np.zeros((128, BF_COLS), np.float32)
        o = 0
        bf[:, o:o + KT * JD] = w.reshape(KT, 128, JD).transpose(1, 0, 2).reshape(128, KT * JD); o += KT * JD
        bf[:, o] = 0.125; o += 1
        bf[:, o:o + KT * B] = xt.reshape(KT, 128, B).transpose(1, 0, 2).reshape(128, KT * B); o += KT * B
        bf[:, o:o + KL] = xb[0:128]; o += KL
        bf[:, o:o + KL] = xb[128:256]; o += KL
        assert o == BF_COLS
        f8b = np.zeros((128, F8_COLS), np.float32)
        f8b[:, 0:F8_XO] = xown.reshape(KTF, 128, BL).transpose(1, 0, 2).reshape(128, KTF * BL)
        f8b[:, F8_XO:F8_COLS] = wif_t
        in_maps.append({
            "bfin": bf.astype(ml_dtypes.bfloat16),
            "f8in": f8b.astype(ml_dtypes.bfloat16),
            "f32in": f32_blob,
        })
    return in_maps


def run(x, W, trace=False):
    from concourse.bass_utils import run_bass_kernel_spmd

    if "nc" not in _CACHE:
        _CACHE["nc"] = _build_module()
    nc = _CACHE["nc"]
    in_maps = _prep_in_maps(x, W)
    res = run_bass_kernel_spmd(
        nc, in_maps, core_ids=list(range(NCORES)), trace=trace
    )
    v = np.concatenate(
        [np.asarray(res.results[c]["out"], np.float32) for c in range(NCORES)],
        axis=0,
    )                                                               # (B, (d,j))
    out = v.reshape(B, D, J).transpose(0, 2, 1)[..., None]
    return np.ascontiguousarray(out.astype(np.float32)), res


def kernel(x, W):
    out, _ = run(x, W, trace=False)
    return out


# revision 15
# speedup vs baseline: 1.0019x; 1.0019x over previous
"""CapsuleLayer dynamic-routing kernel for 8 TRN2 NeuronCores.

Sharding: in_size (i) is split 8 ways (144 rows/core) for routing iterations
2-3; iteration 1 is batch-sharded.  u_hat (B,1152,10,16 = 189MB) is never
materialized: both the c-weighted sum (s_j) and the agreement update factor
through x and W:

    s_un[b, (d,j)]   = sum_{(i,u)} x[b,u,i] * (e[i,j] * W[i,j,d,u])
    A[(i,u), (d,j)]  = sum_b x[b,u,i] * v[b,j,d]
    u_vj1[i,j]       = (1/B) sum_{u,d} W[i,j,d,u] * A[(i,u),(d,j)]

Collective structure (the cost floor here is the per-collective constant, so
count is everything): iteration 1's coupling weights are exactly uniform
(softmax of ones), so s1 = x . (W/I) is a fixed linear map -- each core
computes s1 and v1 for its OWN 32 batches over the full contraction with a
prebaked W/I, and the only cross-core exchange is an AllGather of v1 (no
1.875x AllReduce tax).  Iterations 2-3 run i-sharded as before: AllReduce of
s2 (with the softmax denominator z riding in column 160), ReduceScatter of
s3, and each core squashes/emits only its own 32-batch output shard (gathered
host-side).  Key layouts: contraction index is (i*8+u) on partitions; the
160-wide capsule axis is d-major (d*10+j); s is produced in (b,(d,j))
orientation so squash needs no cross-partition reduction and v feeds the
A-matmul with no transposes; per-i-block sums/broadcasts (u_vj1, 1/z) are
constant 0/1-pattern matmuls.  All matmuls run in bf16 with fp32 PSUM
accumulation; exp/ln/copy/square stay in one ACT table so only one
LoadActFuncSet is ever issued.
"""

import os
import sys

import numpy as np

for _p in ("/opt/trn_rl_repo",):
    if _p not in sys.path and os.path.isdir(_p):
        sys.path.insert(0, _p)

import ml_dtypes

NCORES = 8
B, U, I = 256, 8, 1152
J, D = 10, 16
IL = I // NCORES        # 144 in_size rows per core
KL = IL * U             # 1152 local contraction length (i,u)
KT = KL // 128          # 9 partition tiles
JD = J * D              # 160
KTF = (I * U) // 128    # 72 full-contraction tiles (batch-sharded front)
BL = B // NCORES        # 32 own batches
W_END = KT * JD + 1
XT_END = W_END + KT * B
BF_COLS = XT_END + 2 * KL
F8_XO = KTF * BL
F8_COLS = F8_XO + KTF * JD                         # x-own | W-full (fp8 front)
F32_COLS = 256 + JD + 1                            # m8 | ones | sel10 | -ln(I)

_CACHE = {}


def _build_module():
    import concourse.bacc as bacc
    import concourse.mybir as mybir
    import concourse.tile as tile

    f32 = mybir.dt.float32
    bf16 = mybir.dt.bfloat16
    AF = mybir.ActivationFunctionType
    ALU = mybir.AluOpType
    AX = mybir.AxisListType

    # Force the act-table pass's first-match lookup to land every function
    # we use (Exp, Ln, Copy, Square) on the one table that covers them all,
    # so only a single LoadActFuncSet is ever emitted.  Table *ids* are
    # positional, so we only hide functions from other tables, never reorder.
    import concourse.hw_specs as hw_specs
    if not hasattr(bacc, "_orig_get_activation_tables"):
        bacc._orig_get_activation_tables = bacc.get_activation_tables

        def _patched_tables(arch):
            tabs = bacc._orig_get_activation_tables(arch)
            AF_ = mybir.ActivationFunctionType
            ours = {AF_.Exp, AF_.Ln, AF_.Copy, AF_.Square, AF_.Identity}
            out = {}
            for name, s in tabs.items():
                if name == "natural_log_exp_and_others":
                    out[name] = s
                else:
                    out[name] = s - ours
            return out

        bacc.get_activation_tables = _patched_tables

    nc = bacc.Bacc(
        "TRN2", target_bir_lowering=False, debug=False, num_devices=NCORES
    )

    bf_d = nc.declare_dram_parameter("bfin", [128, BF_COLS], bf16, isOutput=False)
    f8_d = nc.declare_dram_parameter("f8in", [128, F8_COLS], bf16, isOutput=False)
    f32_d = nc.declare_dram_parameter("f32in", [128, F32_COLS], f32, isOutput=False)
    out_d = nc.declare_dram_parameter("out", [B // NCORES, JD], f32, isOutput=True)

    with tile.TileContext(nc) as tc:
        with (
            tc.tile_pool(name="const", bufs=1) as cpool,
            tc.tile_pool(name="work", bufs=2) as wpool,
            tc.tile_pool(name="psum", bufs=1, space="PSUM") as ppool,
            tc.tile_pool(name="apsum", bufs=2, space="PSUM") as apool,
            tc.tile_pool(name="work3", bufs=3) as wpool3,
            tc.tile_pool(name="dram", bufs=3, space="DRAM") as dpool,
        ):
            # ---- persistent loads.  The batch-sharded front's x-own and
            # W-full stream first in fp8 (they gate s1; iteration 1 only sets
            # routing logits, never the output path, so fp8 noise is washed
            # out by the softmax over 1152 rows); the i-shard tensors ride
            # behind on both HWDGE queues and only need to land before the
            # post-AllGather phase. ----
            xo_sb = cpool.tile([128, KTF * BL], bf16)
            nc.sync.dma_start(xo_sb[:, :], f8_d[:, 0:F8_XO])
            xo = xo_sb[:, :].rearrange("p (t b) -> p t b", b=BL)
            wi_sb = cpool.tile([128, KTF * JD], bf16)
            WI_CH = 4
            wi_step = (KTF // WI_CH) * JD
            for ch in range(WI_CH):
                q = nc.scalar if ch % 2 else nc.sync
                q.dma_start(
                    wi_sb[:, ch * wi_step:(ch + 1) * wi_step],
                    f8_d[:, F8_XO + ch * wi_step:F8_XO + (ch + 1) * wi_step],
                )
            wi = wi_sb[:, :].rearrange("p (t n) -> p t n", n=JD)

            wsb_sb = cpool.tile([128, W_END], bf16)
            nc.sync.dma_start(wsb_sb[:, :], bf_d[:, 0:W_END])
            wsb = wsb_sb[:, 0:KT * JD].rearrange("p (t n) -> p t n", n=JD)
            ones8 = wsb_sb[:, KT * JD:W_END]
            f32_sb = cpool.tile([128, F32_COLS], f32)
            nc.scalar.dma_start(f32_sb[:, :], f32_d[:, :])
            xt_sb = cpool.tile([128, KT * B], bf16)
            nc.scalar.dma_start(xt_sb[:, :], bf_d[:, W_END:XT_END])
            xt = xt_sb[:, :].rearrange("p (t b) -> p t b", b=B)
            xb_sb = cpool.tile([128, 2 * KL], bf16)
            nc.sync.dma_start(xb_sb[:, :], bf_d[:, XT_END:BF_COLS])
            xb0 = xb_sb[:, 0:KL]
            xb1 = xb_sb[:, KL:2 * KL]
            m8 = f32_sb[:, 0:128]
            ones10 = f32_sb[0:J, 128:256]     # (10, 128) of ones
            sel10 = f32_sb[0:J, 256:256 + JD]  # sel10[j', d*J+j] = (j==j')
            negln_i = f32_sb[:, 256 + JD:256 + JD + 1]

            b_b = cpool.tile([128, KT, J], f32)     # b_ij replicated over u
            nc.vector.memset(b_b[:, :, :], 1.0)

            # PE p-state warm-up: a dependency-free junk-matmul chain keeps
            # the tensor engine continuously busy while the front's W stream
            # lands, so the s1 matmuls dispatch at the ramped clock.  Output
            # goes to the uv PSUM tile, which the real uv matmul later resets
            # with start=True.
            uv_warm = ppool.tile([128, KT * J], f32, tag="uv_ps", name="uv_warm")
            for _wk in range(48):
                nc.tensor.matmul(
                    uv_warm[0:J, :], b_b[:, 0, :],
                    b_b[:, :, :].rearrange("p t j -> p (t j)"),
                    start=True, stop=True,
                )

            def wc_group(wc, e_b, g):
                nc.vector.tensor_tensor(
                    wc[:, 3 * g:3 * (g + 1), :].rearrange("p t (d j) -> p t d j", j=J),
                    wsb[:, 3 * g:3 * (g + 1), :].rearrange("p t (d j) -> p t d j", j=J),
                    e_b[:, 3 * g:3 * (g + 1), :].unsqueeze(2).broadcast_to([128, 3, D, J]),
                    ALU.mult,
                )

            def s_mms(s_ps, wc, t):
                # the two b-halves live in separate PSUM banks: a start=True
                # matmul clears its bank, so interleaved accumulation groups
                # must not share one
                s_ps0, s_ps1 = s_ps
                nc.tensor.matmul(
                    s_ps0[:, :], xt[:, t, 0:128], wc[:, t, :],
                    start=(t == 0), stop=(t == KT - 1),
                )
                nc.tensor.matmul(
                    s_ps1[:, :], xt[:, t, 128:B], wc[:, t, :],
                    start=(t == 0), stop=(t == KT - 1),
                )

            def z_mms(e_b):
                # z_loc[j] = sum_i e[i,j] = (1/8)*sum_partitions e_b, as a
                # (J,1) column; also keeps the PE warm between phases
                z_ps = ppool.tile([J, 1], f32, tag="z_ps")
                for t in range(KT):
                    nc.tensor.matmul(
                        z_ps[:, :], e_b[:, t, :], ones8[:, 0:1],
                        start=(t == 0), stop=(t == KT - 1),
                    )
                return z_ps

            def stage_and_collect(s_ps, z_ps, last):
                # stage [s | z] in SBUF: the z column is written into the
                # right partition blocks with tiny DVE copies so the two wide
                # DMAs carry everything; PSUM itself is not DMA-readable
                s_ps0, s_ps1 = s_ps
                cdt = f32 if last else bf16
                s_sb = wpool.tile([128, 2 * (JD + 1)], cdt, tag="s_sb")
                nc.scalar.copy(s_sb[:, 0:JD], s_ps0[:, :])
                nc.vector.tensor_copy(s_sb[:, JD + 1:2 * JD + 1], s_ps1[:, :])
                for r in range(4) if last else range(1):
                    nc.vector.tensor_copy(
                        s_sb[r * 32:r * 32 + J, JD:JD + 1], z_ps[:, :]
                    )
                    if last:
                        nc.vector.tensor_copy(
                            s_sb[r * 32:r * 32 + J, 2 * JD + 1:2 * JD + 2], z_ps[:, :]
                        )
                cc_in = dpool.tile([B, JD + 1], cdt, tag="cc_in")
                nc.sync.dma_start(
                    cc_in[:, :].rearrange("(c p) n -> p c n", p=128),
                    s_sb[:, :].rearrange("p (c n) -> p c n", n=JD + 1),
                )
                kind = "ReduceScatter" if last else "AllReduce"
                shape = [B // NCORES, JD + 1] if last else [B, JD + 1]
                cc_out = dpool.tile(shape, cdt, tag="cc3_out" if last else "cc_out", name="ccout")
                nc.gpsimd.collective_compute(
                    kind,
                    ALU.add,
                    replica_groups=[list(range(NCORES))],
                    ins=[cc_in.opt()],
                    outs=[cc_out.opt()],
                )
                return cc_out

            # ---- iteration 1, batch-sharded: c1 is exactly uniform (softmax
            # of ones), so s1 = x . (W/I) over the full (i,u) contraction for
            # this core's own 32 batches -- no exp, no z, no collective ----
            s1_ps = ppool.tile([BL, JD], f32, tag="s1_ps")
            for t in range(KTF):
                nc.tensor.matmul(
                    s1_ps[:, :], xo[:, t, :], wi[:, t, :],
                    start=(t == 0), stop=(t == KTF - 1),
                )
            # s1 = s_raw / I with the 1/I folded into the squash: the square
            # gets scale=1/I, and sqrt(m)/I = exp(0.5*ln(m) - ln(I)) folds the
            # remaining factor into the Exp bias -- zero extra instructions
            sq1 = wpool.tile([BL, JD], bf16, tag="sq1")
            nc.scalar.activation(sq1[:, :], s1_ps[:, :], AF.Square, scale=1.0 / I)
            msq1 = wpool.tile([BL, D], f32, tag="msq1")
            nc.vector.tensor_reduce(
                msq1[:, :],
                sq1[:, :].rearrange("p (d j) -> p d j", j=J),
                axis=AX.X,
                op=ALU.add,
            )
            ln1 = wpool.tile([BL, D], f32, tag="ln1")
            nc.scalar.activation(ln1[:, :], msq1[:, :], AF.Ln)
            rt1 = wpool.tile([BL, D], f32, tag="rt1")
            nc.scalar.activation(
                rt1[:, :], ln1[:, :], AF.Exp, scale=0.5, bias=negln_i[0:BL, 0:1]
            )
            dn1 = wpool.tile([BL, D], f32, tag="dn1")
            nc.vector.tensor_scalar_add(dn1[:, :], msq1[:, :], 1.0)
            rc1 = wpool.tile([BL, D], f32, tag="rc1")
            nc.vector.reciprocal(rc1[:, :], dn1[:, :])
            f1 = wpool.tile([BL, D], f32, tag="f1")
            nc.vector.tensor_mul(f1[:, :], rt1[:, :], rc1[:, :])
            v1 = wpool.tile([BL, JD], bf16, tag="v1")
            nc.vector.tensor_tensor(
                v1[:, :].rearrange("p (d j) -> p d j", j=J),
                s1_ps[:, :].rearrange("p (d j) -> p d j", j=J),
                f1[:, :].unsqueeze(2).broadcast_to([BL, D, J]),
                ALU.mult,
            )
            ag_in = dpool.tile([BL, JD], bf16, tag="ag_in")
            nc.sync.dma_start(ag_in[:, :], v1[:, :])
            ag_out = dpool.tile([B, JD], bf16, tag="ag_out", name="agout")
            nc.gpsimd.collective_compute(
                "AllGather",
                ALU.bypass,
                replica_groups=[list(range(NCORES))],
                ins=[ag_in.opt()],
                outs=[ag_out.opt()],
            )

            cc_out = None
            for it in range(2):
                last_cc = it == 1

                if it == 0:
                    # v1 arrives whole from the AllGather, already in the
                    # (b-on-partitions, (d,j)) layout the A-matmul wants
                    vt = wpool.tile([128, 2 * JD], bf16, tag="vt")
                    nc.sync.dma_start(
                        vt[:, :].rearrange("p (c n) -> p c n", n=JD),
                        ag_out[:, :].rearrange("(c p) n -> p c n", p=128),
                    )
                else:
                    # ---- post-AllReduce squash -> v ----
                    sgz = wpool.tile([128, 2 * (JD + 1)], bf16, tag="sgz")
                    nc.sync.dma_start(
                        sgz[:, :].rearrange("p (c n) -> p c n", n=JD + 1),
                        cc_out[:, :].rearrange("(c p) n -> p c n", p=128),
                    )
                    sg = sgz[:, :].rearrange("p (c n) -> p c n", n=JD + 1)[:, :, 0:JD]

                    # zinv at (d,j) columns on all 128 partitions: recip the z
                    # column, scale sel10 by it per-partition, then a (K=10)
                    # ones-matmul lifts it to 128 partitions
                    s_n = wpool.tile([128, 2 * JD], f32, tag="s_n")
                    zinv = wpool.tile([J, 1], f32, tag="zinv")
                    nc.vector.reciprocal(zinv[:, :], sgz[0:J, JD:JD + 1])
                    zsel = wpool.tile([J, JD], f32, tag="zsel")
                    nc.vector.tensor_scalar_mul(zsel[:, :], sel10[:, :], zinv[:, 0:1])
                    zbc_ps = ppool.tile([128, JD], f32, tag="zbc_ps")
                    nc.tensor.matmul(zbc_ps[:, :], ones10[:, :], zsel[:, :], start=True, stop=True)

                    # s = s_un / z[j]
                    nc.vector.tensor_tensor(
                        s_n[:, :].rearrange("p (c n) -> p c n", n=JD),
                        sg,
                        zbc_ps[:, :].unsqueeze(1).broadcast_to([128, 2, JD]),
                        ALU.mult,
                    )

                    # mag_sq[b, d] = sum_j s[b, (d,j)]^2 : plain innermost reduce
                    sq = wpool.tile([128, 2 * JD], bf16, tag="sq")
                    nc.scalar.square(sq[:, :], s_n[:, :])
                    msq = wpool.tile([128, 2 * D], f32, tag="msq")
                    nc.vector.tensor_reduce(
                        msq[:, :].rearrange("p (c d) -> p c d", d=D),
                        sq[:, :].rearrange("p (c d j) -> p c d j", d=D, j=J),
                        axis=AX.X,
                        op=ALU.add,
                    )

                    # F = sqrt(m)/(1+m) on the tiny (128, 32) footprint;
                    # sqrt(m) = exp(0.5*ln(m)) keeps every ACT func in one table
                    lnm = wpool.tile([128, 2 * D], f32, tag="lnm")
                    nc.scalar.activation(lnm[:, :], msq[:, :], AF.Ln)
                    rt = wpool.tile([128, 2 * D], f32, tag="rt")
                    nc.scalar.activation(rt[:, :], lnm[:, :], AF.Exp, scale=0.5)
                    dn = wpool.tile([128, 2 * D], f32, tag="dn")
                    nc.vector.tensor_scalar_add(dn[:, :], msq[:, :], 1.0)
                    rc = wpool.tile([128, 2 * D], f32, tag="rc")
                    nc.vector.reciprocal(rc[:, :], dn[:, :])
                    f_t = wpool.tile([128, 2 * D], f32, tag="f_t")
                    nc.vector.tensor_mul(f_t[:, :], rt[:, :], rc[:, :])

                    # v = s * F (F broadcast over j); v lands directly in the
                    # (b, (d,j)) layout the A-matmul needs -- no transposes
                    vt = wpool.tile([128, 2 * JD], bf16, tag="vt")
                    for ch in range(2):
                        nc.vector.tensor_tensor(
                            vt[:, ch * JD:(ch + 1) * JD].rearrange("p (d j) -> p d j", j=J),
                            s_n[:, ch * JD:(ch + 1) * JD].rearrange("p (d j) -> p d j", j=J),
                            f_t[:, ch * D:(ch + 1) * D].unsqueeze(2).broadcast_to([128, D, J]),
                            ALU.mult,
                        )
                vb0 = vt[:, 0:JD]
                vb1 = vt[:, JD:2 * JD]

                # ---- fused: A-path group g immediately feeds the next
                # iteration's e/Wc/s-matmuls for those k-tiles ----
                e_b = wpool.tile([128, KT, J], bf16, tag="e_b")
                wc = wpool.tile([128, KT, JD], bf16, tag="wc")
                s_ps = (
                    ppool.tile([128, JD], f32, tag="s_ps0", name="s_ps0"),
                    ppool.tile([128, JD], f32, tag="s_ps1", name="s_ps1"),
                )
                r_t = wpool.tile([128, KT, J], f32, tag="r_t")
                uv_ps = ppool.tile([128, KT * J], f32, tag="uv_ps")
                for g in range(3):
                    a_ps = apool.tile([128, 3 * JD], f32, tag="a_ps")
                    for tt in range(3):
                        t = g * 3 + tt
                        nc.tensor.matmul(
                            a_ps[:, tt * JD:(tt + 1) * JD],
                            xb0[:, t * 128:(t + 1) * 128], vb0,
                            start=True, stop=False,
                        )
                        nc.tensor.matmul(
                            a_ps[:, tt * JD:(tt + 1) * JD],
                            xb1[:, t * 128:(t + 1) * 128], vb1,
                            start=False, stop=True,
                        )
                    a_sb = wpool3.tile([128, 3 * JD], bf16, tag="a_sb")
                    nc.scalar.copy(a_sb[:, :], a_ps[:, :])
                    p_t = wpool3.tile([128, 3 * JD], bf16, tag="p_t")
                    nc.vector.tensor_tensor(
                        p_t[:, :],
                        wsb[:, g * 3:(g + 1) * 3, :].rearrange("p a n -> p (a n)"),
                        a_sb[:, :],
                        ALU.mult,
                    )
                    nc.vector.tensor_reduce(
                        r_t[:, g * 3:(g + 1) * 3, :],
                        p_t.rearrange("p (a d j) -> p a j d", d=D, j=J),
                        axis=AX.X,
                        op=ALU.add,
                    )
                # single 90-column matmul: uv[(i,u-blocks), (t j)] via the
                # constant block-averaging matrix m8
                nc.tensor.matmul(
                    uv_ps[:, :], m8[:, :],
                    r_t[:, :, :].rearrange("p t j -> p (t j)"),
                    start=True, stop=True,
                )
                nc.vector.tensor_add(
                    b_b[:, :, :], b_b[:, :, :],
                    uv_ps.rearrange("p (t j) -> p t j", j=J),
                )
                nc.scalar.activation(e_b[:, :, :], b_b[:, :, :], AF.Exp)
                for g in range(3):
                    wc_group(wc, e_b, g)
                for t in range(KT):
                    s_mms(s_ps, wc, t)
                cc_out = stage_and_collect(s_ps, z_mms(e_b), last=last_cc)

            # ---- post-ReduceScatter shard squash -> out ----
            sg3z = wpool.tile([32, JD + 1], f32, tag="sg3z")
            nc.sync.dma_start(sg3z[:, :], cc_out[0:32, 0:JD + 1])
            zinv3 = wpool.tile([J, 1], f32, tag="zinv3")
            nc.vector.reciprocal(zinv3[:, :], sg3z[0:J, JD:JD + 1])
            zsel3 = wpool.tile([J, JD], f32, tag="zsel3")
            nc.vector.tensor_scalar_mul(zsel3[:, :], sel10[:, :], zinv3[:, 0:1])
            zbc3 = ppool.tile([32, JD], f32, tag="zbc_ps", name="zbc3")
            nc.tensor.matmul(zbc3[:, :], ones10[:, 0:32], zsel3[:, :], start=True, stop=True)
            sn3 = wpool.tile([32, JD], f32, tag="sn3")
            nc.vector.tensor_mul(sn3[:, :], sg3z[0:32, 0:JD], zbc3[:, :])
            sq3 = wpool.tile([32, JD], bf16, tag="sq3")
            nc.scalar.square(sq3[:, :], sn3[:, :])
            msq3 = wpool.tile([32, D], f32, tag="msq3")
            nc.vector.tensor_reduce(
                msq3[:, :],
                sq3[:, :].rearrange("p (d j) -> p d j", j=J),
                axis=AX.X,
                op=ALU.add,
            )
            ln3 = wpool.tile([32, D], f32, tag="ln3")
            nc.scalar.activation(ln3[:, :], msq3[:, :], AF.Ln)
            rt3 = wpool.tile([32, D], f32, tag="rt3")
            nc.scalar.activation(rt3[:, :], ln3[:, :], AF.Exp, scale=0.5)
            dn3 = wpool.tile([32, D], f32, tag="dn3")
            nc.vector.tensor_scalar_add(dn3[:, :], msq3[:, :], 1.0)
            rc3 = wpool.tile([32, D], f32, tag="rc3")
            nc.vector.reciprocal(rc3[:, :], dn3[:, :])
            f3 = wpool.tile([32, D], f32, tag="f3")
            nc.vector.tensor_mul(f3[:, :], rt3[:, :], rc3[:, :])
            v3 = wpool.tile([32, JD], f32, tag="v3")
            nc.vector.tensor_tensor(
                v3[:, :].rearrange("p (d j) -> p d j", j=J),
                sn3[:, :].rearrange("p (d j) -> p d j", j=J),
                f3[:, :].unsqueeze(2).broadcast_to([32, D, J]),
                ALU.mult,
            )
            nc.sync.dma_start(out_d[:, :], v3[:, :])

    nc.finalize()
    return nc


def _f32_blob():
    blob = np.zeros((128, F32_COLS), np.float32)
    blob[:, 0:128] = np.kron(np.eye(16, dtype=np.float32), np.ones((8, 8), np.float32)) / B
    blob[0:J, 128:256] = 1.0
    blob[0:J, 256:256 + JD] = np.tile(np.eye(J, dtype=np.float32), (1, D))
    blob[:, 256 + JD] = -np.log(float(I))
    return blob


def _prep_in_maps(x, W):
    x = np.asarray(x, np.float32)
    W = np.asarray(W, np.float32)
    Wm = W[0]
    f32_blob = _f32_blob()
    # full-contraction W for the batch-sharded first iteration (the 1/I is
    # folded into the on-core squash so fp8 doesn't underflow), shared by all
    # cores: rows (i*8+u), cols (d*10+j)
    wif = Wm.transpose(0, 3, 2, 1).reshape(I * U, JD)
    wif_t = wif.reshape(KTF, 128, JD).transpose(1, 0, 2).reshape(128, KTF * JD)
    in_maps = []
    for c in range(NCORES):
        sl = slice(c * IL, (c + 1) * IL)
        xs = x[:, :, sl]                                            # (B, U, IL)
        xt = np.ascontiguousarray(xs.transpose(2, 1, 0).reshape(KL, B))
        xb = xt.T
        w = Wm[sl].transpose(0, 3, 2, 1).reshape(KL, JD)   # cols = (d, j)
        xown = x[c * BL:(c + 1) * BL].transpose(2, 1, 0).reshape(I * U, BL)
        bf = np.zeros((128, BF_COLS), np.float32)
        o = 0
        bf[:, o:o + KT * JD] = w.reshape(KT, 128, JD).transpose(1, 0, 2).reshape(128, KT * JD); o += KT * JD
        bf[:, o] = 0.125; o += 1
        bf[:, o:o + KT * B] = xt.reshape(KT, 128, B).transpose(1, 0, 2).reshape(128, KT * B); o += KT * B
        bf[:, o:o + KL] = xb[0:128]; o += KL
        bf[:, o:o + KL] = xb[128:256]; o += KL
        assert o == BF_COLS
        f8b = np.zeros((128, F8_COLS), np.float32)
        f8b[:, 0:F8_XO] = xown.reshape(KTF, 128, BL).transpose(1, 0, 2).reshape(128, KTF * BL)
        f8b[:, F8_XO:F8_COLS] = wif_t
        in_maps.append({
            "bfin": bf.astype(ml_dtypes.bfloat16),
            "f8in": f8b.astype(ml_dtypes.bfloat16),
            "f32in": f32_blob,
        })
    return in_maps


def run(x, W, trace=False):
    from concourse.bass_utils import run_bass_kernel_spmd

    if "nc" not in _CACHE:
        _CACHE["nc"] = _build_module()
    nc = _CACHE["nc"]
    in_maps = _prep_in_maps(x, W)
    res = run_bass_kernel_spmd(
        nc, in_maps, core_ids=list(range(NCORES)), trace=trace
    )
    v = np.concatenate(
        [np.asarray(res.results[c]["out"], np.float32) for c in range(NCORES)],
        axis=0,
    )                                                               # (B, (d,j))
    out = v.reshape(B, D, J).transpose(0, 2, 1)[..., None]
    return np.ascontiguousarray(out.astype(np.float32)), res


def kernel(x, W):
    out, _ = run(x, W, trace=False)
    return out


# revision 18
# speedup vs baseline: 1.0491x; 1.0471x over previous
"""CapsuleLayer dynamic-routing kernel for 8 TRN2 NeuronCores.

Sharding: in_size (i) is split 8 ways (144 rows/core) for routing iterations
2-3; iteration 1 is batch-sharded.  u_hat (B,1152,10,16 = 189MB) is never
materialized: both the c-weighted sum (s_j) and the agreement update factor
through x and W:

    s_un[b, (d,j)]   = sum_{(i,u)} x[b,u,i] * (e[i,j] * W[i,j,d,u])
    A[(i,u), (d,j)]  = sum_b x[b,u,i] * v[b,j,d]
    u_vj1[i,j]       = (1/B) sum_{u,d} W[i,j,d,u] * A[(i,u),(d,j)]

Collective structure (the cost floor here is the per-collective constant, so
count is everything): iteration 1's coupling weights are exactly uniform
(softmax of ones), so s1 = x . (W/I) is a fixed linear map -- each core
computes s1 and v1 for its OWN 32 batches over the full contraction with a
prebaked W/I, and the only cross-core exchange is an AllGather of v1 (no
1.875x AllReduce tax).  Iterations 2-3 run i-sharded as before: AllReduce of
s2 (with the softmax denominator z riding in column 160), ReduceScatter of
s3, and each core squashes/emits only its own 32-batch output shard (gathered
host-side).  Key layouts: contraction index is (i*8+u) on partitions; the
160-wide capsule axis is d-major (d*10+j); s is produced in (b,(d,j))
orientation so squash needs no cross-partition reduction and v feeds the
A-matmul with no transposes; per-i-block sums/broadcasts (u_vj1, 1/z) are
constant 0/1-pattern matmuls.  All matmuls run in bf16 with fp32 PSUM
accumulation; exp/ln/copy/square stay in one ACT table so only one
LoadActFuncSet is ever issued.
"""

import os
import sys

import numpy as np

for _p in ("/opt/trn_rl_repo",):
    if _p not in sys.path and os.path.isdir(_p):
        sys.path.insert(0, _p)

import ml_dtypes

NCORES = 8
B, U, I = 256, 8, 1152
J, D = 10, 16
IL = I // NCORES        # 144 in_size rows per core
KL = IL * U             # 1152 local contraction length (i,u)
KT = KL // 128          # 9 partition tiles
JD = J * D              # 160
KTF = (I * U) // 128    # 72 full-contraction tiles (batch-sharded front)
BL = B // NCORES        # 32 own batches
W_END = KT * JD + 1
XT_END = W_END + KT * B
BF_COLS = XT_END + 2 * KL
F8_XO = KTF * BL
F8_COLS = F8_XO + KTF * JD                         # x-own | W-full (fp8 front)
F32_COLS = 256 + JD + 1                            # m8 | ones | sel10 | -ln(I)

_CACHE = {}


def _build_module():
    import concourse.bacc as bacc
    import concourse.mybir as mybir
    import concourse.tile as tile

    f32 = mybir.dt.float32
    bf16 = mybir.dt.bfloat16
    AF = mybir.ActivationFunctionType
    ALU = mybir.AluOpType
    AX = mybir.AxisListType

    # Force the act-table pass's first-match lookup to land every function
    # we use (Exp, Ln, Copy, Square) on the one table that covers them all,
    # so only a single LoadActFuncSet is ever emitted.  Table *ids* are
    # positional, so we only hide functions from other tables, never reorder.
    import concourse.hw_specs as hw_specs
    if not hasattr(bacc, "_orig_get_activation_tables"):
        bacc._orig_get_activation_tables = bacc.get_activation_tables

        def _patched_tables(arch):
            tabs = bacc._orig_get_activation_tables(arch)
            AF_ = mybir.ActivationFunctionType
            ours = {AF_.Exp, AF_.Ln, AF_.Copy, AF_.Square, AF_.Identity}
            out = {}
            for name, s in tabs.items():
                if name == "natural_log_exp_and_others":
                    out[name] = s
                else:
                    out[name] = s - ours
            return out

        bacc.get_activation_tables = _patched_tables

    nc = bacc.Bacc(
        "TRN2", target_bir_lowering=False, debug=False, num_devices=NCORES
    )

    f8 = mybir.dt.float8e4
    bf_d = nc.declare_dram_parameter("bfin", [128, BF_COLS], bf16, isOutput=False)
    f8_d = nc.declare_dram_parameter("f8in", [128, F8_COLS], f8, isOutput=False)
    f32_d = nc.declare_dram_parameter("f32in", [128, F32_COLS], f32, isOutput=False)
    out_d = nc.declare_dram_parameter("out", [B // NCORES, JD], f32, isOutput=True)

    with tile.TileContext(nc) as tc:
        with (
            tc.tile_pool(name="const", bufs=1) as cpool,
            tc.tile_pool(name="work", bufs=2) as wpool,
            tc.tile_pool(name="psum", bufs=1, space="PSUM") as ppool,
            tc.tile_pool(name="apsum", bufs=2, space="PSUM") as apool,
            tc.tile_pool(name="work3", bufs=3) as wpool3,
            tc.tile_pool(name="dram", bufs=3, space="DRAM") as dpool,
        ):
            # ---- persistent loads.  The batch-sharded front's x-own and
            # W-full stream first in fp8 (they gate s1; iteration 1 only sets
            # routing logits, never the output path, so fp8 noise is washed
            # out by the softmax over 1152 rows); the i-shard tensors ride
            # behind on both HWDGE queues and only need to land before the
            # post-AllGather phase. ----
            xo_sb = cpool.tile([128, KTF * BL], f8)
            nc.sync.dma_start(xo_sb[:, :], f8_d[:, 0:F8_XO])
            xo = xo_sb[:, :].rearrange("p (t b) -> p t b", b=BL)
            wi_sb = cpool.tile([128, KTF * JD], f8)
            WI_CH = 4
            wi_step = (KTF // WI_CH) * JD
            for ch in range(WI_CH):
                q = nc.scalar if ch % 2 else nc.sync
                q.dma_start(
                    wi_sb[:, ch * wi_step:(ch + 1) * wi_step],
                    f8_d[:, F8_XO + ch * wi_step:F8_XO + (ch + 1) * wi_step],
                )
            wi = wi_sb[:, :].rearrange("p (t n) -> p t n", n=JD)

            wsb_sb = cpool.tile([128, W_END], bf16)
            nc.sync.dma_start(wsb_sb[:, :], bf_d[:, 0:W_END])
            wsb = wsb_sb[:, 0:KT * JD].rearrange("p (t n) -> p t n", n=JD)
            ones8 = wsb_sb[:, KT * JD:W_END]
            f32_sb = cpool.tile([128, F32_COLS], f32)
            nc.scalar.dma_start(f32_sb[:, :], f32_d[:, :])
            xt_sb = cpool.tile([128, KT * B], bf16)
            nc.scalar.dma_start(xt_sb[:, :], bf_d[:, W_END:XT_END])
            xt = xt_sb[:, :].rearrange("p (t b) -> p t b", b=B)
            xb_sb = cpool.tile([128, 2 * KL], bf16)
            nc.sync.dma_start(xb_sb[:, :], bf_d[:, XT_END:BF_COLS])
            xb0 = xb_sb[:, 0:KL]
            xb1 = xb_sb[:, KL:2 * KL]
            m8 = f32_sb[:, 0:128]
            ones10 = f32_sb[0:J, 128:256]     # (10, 128) of ones
            sel10 = f32_sb[0:J, 256:256 + JD]  # sel10[j', d*J+j] = (j==j')
            negln_i = f32_sb[:, 256 + JD:256 + JD + 1]

            b_b = cpool.tile([128, KT, J], f32)     # b_ij replicated over u
            nc.vector.memset(b_b[:, :, :], 1.0)

            def wc_group(wc, e_b, g):
                nc.vector.tensor_tensor(
                    wc[:, 3 * g:3 * (g + 1), :].rearrange("p t (d j) -> p t d j", j=J),
                    wsb[:, 3 * g:3 * (g + 1), :].rearrange("p t (d j) -> p t d j", j=J),
                    e_b[:, 3 * g:3 * (g + 1), :].unsqueeze(2).broadcast_to([128, 3, D, J]),
                    ALU.mult,
                )

            def s_mms(s_ps, wc, t):
                # the two b-halves live in separate PSUM banks: a start=True
                # matmul clears its bank, so interleaved accumulation groups
                # must not share one
                s_ps0, s_ps1 = s_ps
                nc.tensor.matmul(
                    s_ps0[:, :], xt[:, t, 0:128], wc[:, t, :],
                    start=(t == 0), stop=(t == KT - 1),
                )
                nc.tensor.matmul(
                    s_ps1[:, :], xt[:, t, 128:B], wc[:, t, :],
                    start=(t == 0), stop=(t == KT - 1),
                )

            def z_mms(e_b):
                # z_loc[j] = sum_i e[i,j] = (1/8)*sum_partitions e_b, as a
                # (J,1) column; also keeps the PE warm between phases
                z_ps = ppool.tile([J, 1], f32, tag="z_ps")
                for t in range(KT):
                    nc.tensor.matmul(
                        z_ps[:, :], e_b[:, t, :], ones8[:, 0:1],
                        start=(t == 0), stop=(t == KT - 1),
                    )
                return z_ps

            def stage_and_collect(s_ps, z_ps, last):
                # stage [s | z] in SBUF: the z column is written into the
                # right partition blocks with tiny DVE copies so the two wide
                # DMAs carry everything; PSUM itself is not DMA-readable
                s_ps0, s_ps1 = s_ps
                cdt = f32 if last else bf16
                s_sb = wpool.tile([128, 2 * (JD + 1)], cdt, tag="s_sb")
                nc.scalar.copy(s_sb[:, 0:JD], s_ps0[:, :])
                nc.vector.tensor_copy(s_sb[:, JD + 1:2 * JD + 1], s_ps1[:, :])
                for r in range(4) if last else range(1):
                    nc.vector.tensor_copy(
                        s_sb[r * 32:r * 32 + J, JD:JD + 1], z_ps[:, :]
                    )
                    if last:
                        nc.vector.tensor_copy(
                            s_sb[r * 32:r * 32 + J, 2 * JD + 1:2 * JD + 2], z_ps[:, :]
                        )
                cc_in = dpool.tile([B, JD + 1], cdt, tag="cc_in")
                nc.sync.dma_start(
                    cc_in[:, :].rearrange("(c p) n -> p c n", p=128),
                    s_sb[:, :].rearrange("p (c n) -> p c n", n=JD + 1),
                )
                kind = "ReduceScatter" if last else "AllReduce"
                shape = [B // NCORES, JD + 1] if last else [B, JD + 1]
                cc_out = dpool.tile(shape, cdt, tag="cc3_out" if last else "cc_out", name="ccout")
                nc.gpsimd.collective_compute(
                    kind,
                    ALU.add,
                    replica_groups=[list(range(NCORES))],
                    ins=[cc_in.opt()],
                    outs=[cc_out.opt()],
                )
                return cc_out

            # ---- iteration 1, batch-sharded: c1 is exactly uniform (softmax
            # of ones), so s1 = x . (W/I) over the full (i,u) contraction for
            # this core's own 32 batches -- no exp, no z, no collective ----
            # four PSUM accumulators so back-to-back matmuls never chain on
            # the same bank's accumulation pipeline (~240ns drain per matmul
            # otherwise); the banks are tags whose next real use starts with
            # start=True, which resets them
            s1_banks = [
                ppool.tile([BL, JD], f32, tag=tg, name=f"s1b{i}")
                for i, tg in enumerate(("s1_ps", "zbc_ps", "s_ps0", "s_ps1"))
            ]
            for t in range(KTF):
                nc.tensor.matmul(
                    s1_banks[t % 4][:, :], xo[:, t, :], wi[:, t, :],
                    start=(t < 4), stop=(t >= KTF - 4),
                )
            s0c = wpool.tile([BL, JD], f32, tag="s0c")
            nc.scalar.copy(s0c[:, :], s1_banks[0][:, :])
            s2c = wpool.tile([BL, JD], f32, tag="s2c")
            nc.vector.tensor_copy(s2c[:, :], s1_banks[2][:, :])
            s01 = wpool.tile([BL, JD], f32, tag="s01")
            nc.vector.tensor_tensor(s01[:, :], s0c[:, :], s1_banks[1][:, :], ALU.add)
            s23 = wpool.tile([BL, JD], f32, tag="s23")
            nc.vector.tensor_tensor(s23[:, :], s2c[:, :], s1_banks[3][:, :], ALU.add)
            s1_ps = wpool.tile([BL, JD], f32, tag="s1t")
            nc.vector.tensor_tensor(s1_ps[:, :], s01[:, :], s23[:, :], ALU.add)
            # s1 = s_raw / I with the 1/I folded into the squash: the square
            # gets scale=1/I, and sqrt(m)/I = exp(0.5*ln(m) - ln(I)) folds the
            # remaining factor into the Exp bias -- zero extra instructions
            sq1 = wpool.tile([BL, JD], bf16, tag="sq1")
            nc.scalar.activation(sq1[:, :], s1_ps[:, :], AF.Square, scale=1.0 / I)
            msq1 = wpool.tile([BL, D], f32, tag="msq1")
            nc.vector.tensor_reduce(
                msq1[:, :],
                sq1[:, :].rearrange("p (d j) -> p d j", j=J),
                axis=AX.X,
                op=ALU.add,
            )
            ln1 = wpool.tile([BL, D], f32, tag="ln1")
            nc.scalar.activation(ln1[:, :], msq1[:, :], AF.Ln)
            rt1 = wpool.tile([BL, D], f32, tag="rt1")
            nc.scalar.activation(
                rt1[:, :], ln1[:, :], AF.Exp, scale=0.5, bias=negln_i[0:BL, 0:1]
            )
            dn1 = wpool.tile([BL, D], f32, tag="dn1")
            nc.vector.tensor_scalar_add(dn1[:, :], msq1[:, :], 1.0)
            rc1 = wpool.tile([BL, D], f32, tag="rc1")
            nc.vector.reciprocal(rc1[:, :], dn1[:, :])
            f1 = wpool.tile([BL, D], f32, tag="f1")
            nc.vector.tensor_mul(f1[:, :], rt1[:, :], rc1[:, :])
            v1 = wpool.tile([BL, JD], bf16, tag="v1")
            nc.vector.tensor_tensor(
                v1[:, :].rearrange("p (d j) -> p d j", j=J),
                s1_ps[:, :].rearrange("p (d j) -> p d j", j=J),
                f1[:, :].unsqueeze(2).broadcast_to([BL, D, J]),
                ALU.mult,
            )
            ag_in = dpool.tile([BL, JD], bf16, tag="ag_in")
            nc.sync.dma_start(ag_in[:, :], v1[:, :])
            ag_out = dpool.tile([B, JD], bf16, tag="ag_out", name="agout")
            nc.gpsimd.collective_compute(
                "AllGather",
                ALU.bypass,
                replica_groups=[list(range(NCORES))],
                ins=[ag_in.opt()],
                outs=[ag_out.opt()],
            )

            cc_out = None
            for it in range(2):
                last_cc = it == 1

                if it == 0:
                    # v1 arrives whole from the AllGather, already in the
                    # (b-on-partitions, (d,j)) layout the A-matmul wants
                    vt = wpool.tile([128, 2 * JD], bf16, tag="vt")
                    nc.sync.dma_start(
                        vt[:, :].rearrange("p (c n) -> p c n", n=JD),
                        ag_out[:, :].rearrange("(c p) n -> p c n", p=128),
                    )
                else:
                    # ---- post-AllReduce squash -> v ----
                    sgz = wpool.tile([128, 2 * (JD + 1)], bf16, tag="sgz")
                    nc.sync.dma_start(
                        sgz[:, :].rearrange("p (c n) -> p c n", n=JD + 1),
                        cc_out[:, :].rearrange("(c p) n -> p c n", p=128),
                    )
                    sg = sgz[:, :].rearrange("p (c n) -> p c n", n=JD + 1)[:, :, 0:JD]

                    # zinv at (d,j) columns on all 128 partitions: recip the z
                    # column, scale sel10 by it per-partition, then a (K=10)
                    # ones-matmul lifts it to 128 partitions
                    s_n = wpool.tile([128, 2 * JD], f32, tag="s_n")
                    zinv = wpool.tile([J, 1], f32, tag="zinv")
                    nc.vector.reciprocal(zinv[:, :], sgz[0:J, JD:JD + 1])
                    zsel = wpool.tile([J, JD], f32, tag="zsel")
                    nc.vector.tensor_scalar_mul(zsel[:, :], sel10[:, :], zinv[:, 0:1])
                    zbc_ps = ppool.tile([128, JD], f32, tag="zbc_ps")
                    nc.tensor.matmul(zbc_ps[:, :], ones10[:, :], zsel[:, :], start=True, stop=True)

                    # s = s_un / z[j]
                    nc.vector.tensor_tensor(
                        s_n[:, :].rearrange("p (c n) -> p c n", n=JD),
                        sg,
                        zbc_ps[:, :].unsqueeze(1).broadcast_to([128, 2, JD]),
                        ALU.mult,
                    )

                    # mag_sq[b, d] = sum_j s[b, (d,j)]^2 : plain innermost reduce
                    sq = wpool.tile([128, 2 * JD], bf16, tag="sq")
                    nc.scalar.square(sq[:, :], s_n[:, :])
                    msq = wpool.tile([128, 2 * D], f32, tag="msq")
                    nc.vector.tensor_reduce(
                        msq[:, :].rearrange("p (c d) -> p c d", d=D),
                        sq[:, :].rearrange("p (c d j) -> p c d j", d=D, j=J),
                        axis=AX.X,
                        op=ALU.add,
                    )

                    # F = sqrt(m)/(1+m) on the tiny (128, 32) footprint;
                    # sqrt(m) = exp(0.5*ln(m)) keeps every ACT func in one table
                    lnm = wpool.tile([128, 2 * D], f32, tag="lnm")
                    nc.scalar.activation(lnm[:, :], msq[:, :], AF.Ln)
                    rt = wpool.tile([128, 2 * D], f32, tag="rt")
                    nc.scalar.activation(rt[:, :], lnm[:, :], AF.Exp, scale=0.5)
                    dn = wpool.tile([128, 2 * D], f32, tag="dn")
                    nc.vector.tensor_scalar_add(dn[:, :], msq[:, :], 1.0)
                    rc = wpool.tile([128, 2 * D], f32, tag="rc")
                    nc.vector.reciprocal(rc[:, :], dn[:, :])
                    f_t = wpool.tile([128, 2 * D], f32, tag="f_t")
                    nc.vector.tensor_mul(f_t[:, :], rt[:, :], rc[:, :])

                    # v = s * F (F broadcast over j); v lands directly in the
                    # (b, (d,j)) layout the A-matmul needs -- no transposes
                    vt = wpool.tile([128, 2 * JD], bf16, tag="vt")
                    for ch in range(2):
                        nc.vector.tensor_tensor(
                            vt[:, ch * JD:(ch + 1) * JD].rearrange("p (d j) -> p d j", j=J),
                            s_n[:, ch * JD:(ch + 1) * JD].rearrange("p (d j) -> p d j", j=J),
                            f_t[:, ch * D:(ch + 1) * D].unsqueeze(2).broadcast_to([128, D, J]),
                            ALU.mult,
                        )
                vb0 = vt[:, 0:JD]
                vb1 = vt[:, JD:2 * JD]

                # ---- fused: A-path group g immediately feeds the next
                # iteration's e/Wc/s-matmuls for those k-tiles ----
                e_b = wpool.tile([128, KT, J], bf16, tag="e_b")
                wc = wpool.tile([128, KT, JD], bf16, tag="wc")
                s_ps = (
                    ppool.tile([128, JD], f32, tag="s_ps0", name="s_ps0"),
                    ppool.tile([128, JD], f32, tag="s_ps1", name="s_ps1"),
                )
                r_t = wpool.tile([128, KT, J], f32, tag="r_t")
                uv_ps = ppool.tile([128, KT * J], f32, tag="uv_ps")
                for g in range(3):
                    a_ps = apool.tile([128, 3 * JD], f32, tag="a_ps")
                    for tt in range(3):
                        t = g * 3 + tt
                        nc.tensor.matmul(
                            a_ps[:, tt * JD:(tt + 1) * JD],
                            xb0[:, t * 128:(t + 1) * 128], vb0,
                            start=True, stop=False,
                        )
                        nc.tensor.matmul(
                            a_ps[:, tt * JD:(tt + 1) * JD],
                            xb1[:, t * 128:(t + 1) * 128], vb1,
                            start=False, stop=True,
                        )
                    a_sb = wpool3.tile([128, 3 * JD], bf16, tag="a_sb")
                    nc.scalar.copy(a_sb[:, :], a_ps[:, :])
                    p_t = wpool3.tile([128, 3 * JD], bf16, tag="p_t")
                    nc.vector.tensor_tensor(
                        p_t[:, :],
                        wsb[:, g * 3:(g + 1) * 3, :].rearrange("p a n -> p (a n)"),
                        a_sb[:, :],
                        ALU.mult,
                    )
                    nc.vector.tensor_reduce(
                        r_t[:, g * 3:(g + 1) * 3, :],
                        p_t.rearrange("p (a d j) -> p a j d", d=D, j=J),
                        axis=AX.X,
                        op=ALU.add,
                    )
                # single 90-column matmul: uv[(i,u-blocks), (t j)] via the
                # constant block-averaging matrix m8
                nc.tensor.matmul(
                    uv_ps[:, :], m8[:, :],
                    r_t[:, :, :].rearrange("p t j -> p (t j)"),
                    start=True, stop=True,
                )
                nc.vector.tensor_add(
                    b_b[:, :, :], b_b[:, :, :],
                    uv_ps.rearrange("p (t j) -> p t j", j=J),
                )
                nc.scalar.activation(e_b[:, :, :], b_b[:, :, :], AF.Exp)
                for g in range(3):
                    wc_group(wc, e_b, g)
                for t in range(KT):
                    s_mms(s_ps, wc, t)
                cc_out = stage_and_collect(s_ps, z_mms(e_b), last=last_cc)

            # ---- post-ReduceScatter shard squash -> out ----
            sg3z = wpool.tile([32, JD + 1], f32, tag="sg3z")
            nc.sync.dma_start(sg3z[:, :], cc_out[0:32, 0:JD + 1])
            zinv3 = wpool.tile([J, 1], f32, tag="zinv3")
            nc.vector.reciprocal(zinv3[:, :], sg3z[0:J, JD:JD + 1])
            zsel3 = wpool.tile([J, JD], f32, tag="zsel3")
            nc.vector.tensor_scalar_mul(zsel3[:, :], sel10[:, :], zinv3[:, 0:1])
            zbc3 = ppool.tile([32, JD], f32, tag="zbc_ps", name="zbc3")
            nc.tensor.matmul(zbc3[:, :], ones10[:, 0:32], zsel3[:, :], start=True, stop=True)
            sn3 = wpool.tile([32, JD], f32, tag="sn3")
            nc.vector.tensor_mul(sn3[:, :], sg3z[0:32, 0:JD], zbc3[:, :])
            sq3 = wpool.tile([32, JD], bf16, tag="sq3")
            nc.scalar.square(sq3[:, :], sn3[:, :])
            msq3 = wpool.tile([32, D], f32, tag="msq3")
            nc.vector.tensor_reduce(
                msq3[:, :],
                sq3[:, :].rearrange("p (d j) -> p d j", j=J),
                axis=AX.X,
                op=ALU.add,
            )
            ln3 = wpool.tile([32, D], f32, tag="ln3")
            nc.scalar.activation(ln3[:, :], msq3[:, :], AF.Ln)
            rt3 = wpool.tile([32, D], f32, tag="rt3")
            nc.scalar.activation(rt3[:, :], ln3[:, :], AF.Exp, scale=0.5)
            dn3 = wpool.tile([32, D], f32, tag="dn3")
            nc.vector.tensor_scalar_add(dn3[:, :], msq3[:, :], 1.0)
            rc3 = wpool.tile([32, D], f32, tag="rc3")
            nc.vector.reciprocal(rc3[:, :], dn3[:, :])
            f3 = wpool.tile([32, D], f32, tag="f3")
            nc.vector.tensor_mul(f3[:, :], rt3[:, :], rc3[:, :])
            v3 = wpool.tile([32, JD], f32, tag="v3")
            nc.vector.tensor_tensor(
                v3[:, :].rearrange("p (d j) -> p d j", j=J),
                sn3[:, :].rearrange("p (d j) -> p d j", j=J),
                f3[:, :].unsqueeze(2).broadcast_to([32, D, J]),
                ALU.mult,
            )
            nc.sync.dma_start(out_d[:, :], v3[:, :])

    nc.finalize()
    return nc


def _f32_blob():
    blob = np.zeros((128, F32_COLS), np.float32)
    blob[:, 0:128] = np.kron(np.eye(16, dtype=np.float32), np.ones((8, 8), np.float32)) / B
    blob[0:J, 128:256] = 1.0
    blob[0:J, 256:256 + JD] = np.tile(np.eye(J, dtype=np.float32), (1, D))
    blob[:, 256 + JD] = -np.log(float(I))
    return blob


def _prep_in_maps(x, W):
    x = np.asarray(x, np.float32)
    W = np.asarray(W, np.float32)
    Wm = W[0]
    f32_blob = _f32_blob()
    # full-contraction W for the batch-sharded first iteration (the 1/I is
    # folded into the on-core squash so fp8 doesn't underflow), shared by all
    # cores: rows (i*8+u), cols (d*10+j)
    wif = Wm.transpose(0, 3, 2, 1).reshape(I * U, JD)
    wif_t = wif.reshape(KTF, 128, JD).transpose(1, 0, 2).reshape(128, KTF * JD)
    in_maps = []
    for c in range(NCORES):
        sl = slice(c * IL, (c + 1) * IL)
        xs = x[:, :, sl]                                            # (B, U, IL)
        xt = np.ascontiguousarray(xs.transpose(2, 1, 0).reshape(KL, B))
        xb = xt.T
        w = Wm[sl].transpose(0, 3, 2, 1).reshape(KL, JD)   # cols = (d, j)
        xown = x[c * BL:(c + 1) * BL].transpose(2, 1, 0).reshape(I * U, BL)
        bf = np.zeros((128, BF_COLS), np.float32)
        o = 0
        bf[:, o:o + KT * JD] = w.reshape(KT, 128, JD).transpose(1, 0, 2).reshape(128, KT * JD); o += KT * JD
        bf[:, o] = 0.125; o += 1
        bf[:, o:o + KT * B] = xt.reshape(KT, 128, B).transpose(1, 0, 2).reshape(128, KT * B); o += KT * B
        bf[:, o:o + KL] = xb[0:128]; o += KL
        bf[:, o:o + KL] = xb[128:256]; o += KL
        assert o == BF_COLS
        f8b = np.zeros((128, F8_COLS), np.float32)
        f8b[:, 0:F8_XO] = xown.reshape(KTF, 128, BL).transpose(1, 0, 2).reshape(128, KTF * BL)
        f8b[:, F8_XO:F8_COLS] = wif_t
        in_maps.append({
            "bfin": bf.astype(ml_dtypes.bfloat16),
            "f8in": f8b.astype(ml_dtypes.float8_e4m3),
            "f32in": f32_blob,
        })
    return in_maps


def run(x, W, trace=False):
    from concourse.bass_utils import run_bass_kernel_spmd

    if "nc" not in _CACHE:
        _CACHE["nc"] = _build_module()
    nc = _CACHE["nc"]
    in_maps = _prep_in_maps(x, W)
    res = run_bass_kernel_spmd(
        nc, in_maps, core_ids=list(range(NCORES)), trace=trace
    )
    v = np.concatenate(
        [np.asarray(res.results[c]["out"], np.float32) for c in range(NCORES)],
        axis=0,
    )                                                               # (B, (d,j))
    out = v.reshape(B, D, J).transpose(0, 2, 1)[..., None]
    return np.ascontiguousarray(out.astype(np.float32)), res


def kernel(x, W):
    out, _ = run(x, W, trace=False)
    return out
